# revision 1
# baseline (speedup 1.0000x reference)
"""Trainium2 Bass kernel for nn_MixedAttention.

Full inputs in, full output out. Sharding: 8 cores = 2 (batch) x 4 (head
pairs). Each core computes 2 global + 2 local heads for one batch element.

Key algebraic rewrite for the local branch:
    lscores = (lq@lk1^T)@(lk1@lk2^T) = lq @ (lk1^T@lk1) @ lk2^T
with M = lk1^T@lk1 a [64,64] matrix -- turns a 2048^3 matmul chain into
two small matmuls plus one S x S matmul (30x less PE work).

Precision/dtype strategy: fp32 matmuls run at ~2 cycles/column on the PE,
float32r (TF32-like, ~13-bit mantissa) at ~1. The local-branch scores are
large (|raw| up to ~2000) and feed exp(), so any input rounding there is
amplified exponentially -> the local score chain (hidden^T, score-side
projections, pass-2 score matmul) stays fp32. Everything whose error is
not exp-amplified runs f32r: global q/k (scores |s|<~5), all value paths,
the probs @ v context matmuls, and the local pass-1 max (only needs ~1 ulp
of exp range).

Layout: scores are computed transposed st[j, i] = K_eff @ Q_eff^T so the
context matmul needs no transposed probs (lhsT = v_nat, rhs = e). v gets
an extra ones column so the softmax denominator falls out of the context
matmul for free. Global heads skip max subtraction entirely (mask folded
into the Exp bias); local heads get an exact row max from a separate
f32r pass in the untransposed orientation (free-dim reduce_max), and the
-max correction rides an extra contraction row (K=65) in pass 2.
"""

import math
import os
import sys

import numpy as np

sys.path.insert(0, "/opt/trn_rl_repo")

B, S, HID, HEAD = 2, 2048, 1024, 64
SC = S // 128  # 16 s-chunks of 128
HC = HID // 128  # 8 hidden chunks
N_CORES = 8
SCALE = 1.0 / math.sqrt(HEAD)

W_NAMES = ["wq", "wk", "wv", "wlq", "wlk1", "wlk2", "wlv"]
F32R_PROJ = {"wq", "wk", "wv"}  # projections written as f32r at the source

_CACHE = {}
LAST_RESULTS = None  # stash of BassKernelResults for test.py profiling


def _build():
    import concourse.mybir as mybir
    import concourse.tile as tile
    from concourse import bacc
    from concourse.masks import make_identity

    f32 = mybir.dt.float32
    f32r = mybir.dt.float32r
    AF = mybir.ActivationFunctionType
    ALU = mybir.AluOpType
    AX = mybir.AxisListType

    nc = bacc.Bacc("TRN2", target_bir_lowering=False, debug=False,
                   enable_asserts=False)

    hid_d = nc.dram_tensor("hid", (HID, S), f32, kind="ExternalInput").ap()
    mask_d = nc.dram_tensor("mask", (S,), f32, kind="ExternalInput").ap()
    w_d = {n: nc.dram_tensor(n, (HID, 128), f32, kind="ExternalInput").ap()
           for n in W_NAMES}
    b_d = {n: nc.dram_tensor("b" + n[1:], (128,), f32,
                             kind="ExternalInput").ap() for n in W_NAMES}
    out_d = nc.dram_tensor("out", (S, 256), f32, kind="ExternalOutput").ap()

    with tile.TileContext(nc) as tc:
        with (
            tc.tile_pool(name="const", bufs=1) as constp,
            tc.tile_pool(name="persist", bufs=1) as pp,
            tc.tile_pool(name="wp_g", bufs=1) as wp_g,
            tc.tile_pool(name="epool", bufs=8) as ep,
            tc.tile_pool(name="opool", bufs=1) as op_,
            tc.tile_pool(name="ps_tr", bufs=2, space="PSUM") as ps_tr,
            tc.tile_pool(name="ps_mm", bufs=4, space="PSUM") as ps_mm,
            tc.tile_pool(name="ps_ctx", bufs=2, space="PSUM") as ps_ctx,
            tc.tile_pool(name="dramp", bufs=2, space="DRAM") as dramp,
        ):
            ident = constp.tile([128, 128], f32, name="ident")
            make_identity(nc, ident)
            identr = constp.tile([128, 128], f32r, name="identr")
            nc.vector.tensor_copy(identr, ident)
            ones_sb = constp.tile([128, SC], f32, name="ones_sb")
            nc.vector.memset(ones_sb, 1.0)
            mask_sb = constp.tile([128, SC], f32, name="mask_sb")
            nc.gpsimd.dma_start(mask_sb,
                                mask_d.rearrange("(c p) -> p c", p=128))
            bias_sb = {}
            for n in W_NAMES:
                t = constp.tile([128, 1], f32, name=f"b_{n}")
                nc.gpsimd.dma_start(t, b_d[n][:, None])
                bias_sb[n] = t

            projT = {n: pp.tile([128, S], f32, name=f"projT_{n}")
                     for n in W_NAMES if n not in F32R_PROJ}

            out_sb = op_.tile([128, SC, 256], f32, name="out_sb")

            # ---------- emission helpers ----------

            def emit_wdma(n, iop):
                wsb = iop.tile([128, HC, 128], f32, tag="w", name=f"w_{n}")
                nc.sync.dma_start(
                    wsb, w_d[n].rearrange("(c p) m -> p c m", p=128))
                return wsb

            def emit_proj_half(n, wsb, hidT, half):
                accs = [ps_mm.tile([128, 512], f32, tag="mm",
                                   name=f"acc{i}") for i in range(2)]
                for hc in range(HC):
                    for ic in range(2):
                        icg = half * 2 + ic
                        nc.tensor.matmul(
                            accs[ic], lhsT=wsb[:, hc],
                            rhs=hidT[:, hc, icg * 512:(icg + 1) * 512],
                            start=(hc == 0), stop=(hc == HC - 1))
                for ic in range(2):
                    icg = half * 2 + ic
                    nc.vector.tensor_scalar_add(
                        projT[n][:, icg * 512:(icg + 1) * 512],
                        accs[ic], bias_sb[n])

            def emit_proj(n, hidT, iop):
                wsb = emit_wdma(n, iop)
                for half in range(2):
                    emit_proj_half(n, wsb, hidT, half)

            def build_vaug(vT, vdt):
                # v natural [s, d] + ones column -> [128, SC, 65] f32r
                idm = identr if vdt == f32r else ident
                base = vT.base_partition()
                idsl = slice(base, base + 64)
                vaug = wp_g.tile([128, SC, 65], f32r, tag="vaug",
                                 name="vaug", bufs=2)
                nc.vector.tensor_copy(vaug[:, :, 64], ones_sb)
                for t in range(SC):
                    pt = ps_tr.tile([128, 128], vdt, tag="tr")
                    nc.tensor.transpose(
                        pt[:, :64], vT[:, t * 128:(t + 1) * 128],
                        idm[idsl, idsl])
                    nc.any.tensor_copy(vaug[:, t, :64], pt[:, :64])
                return vaug

            def attention_ic(head, kT, qT, vaug, is_local, ic):
                # main pass: st -> exp -> ctx (+sums via the ones column),
                # then transpose back and divide by the sums
                csl = slice(head * 64, (head + 1) * 64)
                if True:
                    isl = slice(ic * 512, (ic + 1) * 512)
                    ctx = ps_ctx.tile([65, 512], f32, tag="ctx", name="ctx")

                    def ctx_group(es):
                        for jc, e in es:
                            nc.tensor.matmul(ctx, lhsT=vaug[:, jc], rhs=e,
                                             start=(jc == 0),
                                             stop=(jc == SC - 1))

                    prev = None
                    for jg in range(4):
                        es = []
                        for jj in range(4):
                            jc = jg * 4 + jj
                            jsl = slice(jc * 128, (jc + 1) * 128)
                            st = ps_mm.tile([128, 512], f32, tag="mm",
                                            name="st")
                            nc.tensor.matmul(st, lhsT=kT[:, jsl],
                                             rhs=qT[:, isl],
                                             start=True, stop=True)
                            e = ep.tile([128, 512], f32r, tag="e", name="e")
                            bias = 0.0 if is_local else mask_sb[:, jc:jc + 1]
                            nc.scalar.activation(e, st, AF.Exp, bias=bias,
                                                 scale=SCALE)
                            es.append((jc, e))
                        if prev is not None:
                            ctx_group(prev)
                        prev = es
                    ctx_group(prev)
                    ctx_sbc = wp_g.tile([65, 512], f32, tag="ctx_sbc",
                                        name="ctx_sbc")
                    nc.any.tensor_copy(ctx_sbc, ctx)
                    for tt in range(4):
                        t = ic * 4 + tt
                        pt = ps_tr.tile([128, 128], f32, tag="tr")
                        nc.tensor.transpose(
                            pt[:, :65], ctx_sbc[:, tt * 128:(tt + 1) * 128],
                            ident[:65, :65])
                        rec = wp_g.tile([128, 1], f32, tag="rec", name="rec")
                        nc.vector.reciprocal(rec, pt[:, 64:65])
                        nc.vector.tensor_scalar_mul(
                            out_sb[:, t, csl], pt[:, :64], rec)
                    nc.sync.dma_start(
                        out_d.rearrange("(t p) c -> p t c", p=128)[
                            :, ic * 4:(ic + 1) * 4, csl],
                        out_sb[:, ic * 4:(ic + 1) * 4, csl])

            def local_prep(head, wp):
                hh = head % 2
                rs = slice(hh * 64, (hh + 1) * 64)
                if hh == 0:
                    lqT = projT["wlq"][rs]
                    lk1T = projT["wlk1"][rs]
                else:
                    lqT = wp.tile([64, S], f32, tag="s0l", name="s0l")
                    nc.scalar.copy(lqT, projT["wlq"][rs])
                    lk1T = wp.tile([64, S], f32, tag="s1l", name="s1l")
                    nc.scalar.copy(lk1T, projT["wlk1"][rs])

                # lk1 natural [s, d] via transposes
                lk1nat = wp.tile([128, SC, 64], f32, tag="lk1nat",
                                 name="lk1nat", bufs=2)
                for t in range(SC):
                    pt = ps_tr.tile([128, 128], f32, tag="tr")
                    nc.tensor.transpose(
                        pt[:, :64], lk1T[:, t * 128:(t + 1) * 128],
                        ident[:64, :64])
                    nc.any.tensor_copy(lk1nat[:, t], pt[:, :64])
                # M = lk1^T @ lk1 [64, 64] (symmetric)
                mps = ps_mm.tile([128, 512], f32, tag="mm", name="mps")
                for t in range(SC):
                    nc.tensor.matmul(mps[:64, :64], lhsT=lk1nat[:, t],
                                     rhs=lk1nat[:, t],
                                     start=(t == 0), stop=(t == SC - 1))
                m_sb = wp.tile([64, 64], f32, tag="m_sb", name="m_sb",
                               bufs=2)
                nc.any.tensor_copy(m_sb, mps[:64, :64])
                # qaug rows 0:64 = (lq @ M)^T = M @ lq^T (M symmetric);
                # row 64 filled later with -max
                qaug = wp.tile([65, S], f32, tag="qaug", name="qaug",
                               bufs=2)
                for ic in range(4):
                    mm = ps_mm.tile([128, 512], f32, tag="mm", name="mm")
                    nc.tensor.matmul(mm[:64], lhsT=m_sb,
                                     rhs=lqT[:, ic * 512:(ic + 1) * 512],
                                     start=True, stop=True)
                    nc.any.tensor_copy(qaug[:64, ic * 512:(ic + 1) * 512],
                                       mm[:64])
                # k2aug: rows 0:64 = lk2^T, row 64 = ones
                k2aug = wp.tile([65, S], f32, tag="k2aug", name="k2aug",
                                bufs=2)
                nc.scalar.copy(k2aug[:64, :], projT["wlk2"][rs])
                nc.vector.memset(k2aug[64:65, :], 1.0)
                vaug = build_vaug(projT["wlv"][rs], f32)

                # f32r shadows for pass 1 (max only needs ~1 absolute)
                qaug_r = wp.tile([64, S], f32r, tag="qaug_r", name="qaug_r",
                                 bufs=2)
                nc.scalar.copy(qaug_r, qaug[:64])
                k2aug_r = wp.tile([64, S], f32r, tag="k2aug_r",
                                  name="k2aug_r", bufs=2)
                nc.scalar.copy(k2aug_r, k2aug[:64])
                return dict(qaug=qaug, k2aug=k2aug, vaug=vaug,
                            qaug_r=qaug_r, k2aug_r=k2aug_r)

            def local_pass1(head, hs, wp):
                # pass 1: untransposed s[i, j] blocks; row max via free-dim
                # reduce (independent ops; no serial DVE chain)
                qaug_r, k2aug_r = hs["qaug_r"], hs["k2aug_r"]
                maxneg = wp.tile([128, SC], f32, tag="maxneg", name="maxneg",
                                 bufs=2)
                for t in range(SC):
                    pmax = wp.tile([128, 4], f32, tag="pmax", name="pmax",
                                   bufs=2)
                    for j4 in range(4):
                        st = ps_mm.tile([128, 512], f32, tag="mm", name="st1")
                        nc.tensor.matmul(
                            st, lhsT=qaug_r[:, t * 128:(t + 1) * 128],
                            rhs=k2aug_r[:, j4 * 512:(j4 + 1) * 512],
                            start=True, stop=True)
                        nc.vector.tensor_reduce(pmax[:, j4:j4 + 1], st,
                                                axis=AX.X, op=ALU.max)
                    nc.vector.tensor_reduce(maxneg[:, t:t + 1], pmax,
                                            axis=AX.X, op=ALU.max,
                                            negate=True)
                mscr = dramp.tile([S], f32, tag="mscr", name="mscr")
                nc.sync.dma_start(
                    mscr.rearrange("(t p) -> p t", p=128), maxneg)
                nc.sync.dma_start(hs["qaug"][64:65, :], mscr[None, :])

                        # ---------- phase A: hidden^T, projections, global heads ----
            with (
                tc.tile_pool(name="pp_g", bufs=1) as pp_g,
                tc.tile_pool(name="hidT", bufs=1) as hp,
                tc.tile_pool(name="io", bufs=2) as iop,
            ):
                for n in F32R_PROJ:
                    projT[n] = pp_g.tile([128, S], f32r, name=f"projT_{n}")
                hidT = hp.tile([128, HC, S], f32, name="hidT")
                hid_r = hid_d.rearrange("(c p) s -> p c s", p=128)
                # qkv weights first on the gpsimd queue so the first
                # projection matmuls start as soon as hidT chunk 0 lands
                wsb_g = {}
                for n in ["wq", "wk", "wv"]:
                    wsb_g[n] = iop.tile([128, HC, 128], f32, tag="wg",
                                        name=f"w_{n}")
                    nc.gpsimd.dma_start(
                        wsb_g[n], w_d[n].rearrange("(c p) m -> p c m", p=128))
                for hc in range(HC):
                    eng = nc.sync if hc % 2 == 0 else nc.gpsimd
                    eng.dma_start(hidT[:, hc], hid_r[:, hc])
                for n in ["wq", "wk", "wv"]:
                    for half in range(2):
                        emit_proj_half(n, wsb_g[n], hidT, half)
                gvaug = {}
                for hh in range(2):
                    rs = slice(hh * 64, (hh + 1) * 64)
                    gvaug[hh] = build_vaug(projT["wv"][rs], f32r)
                # interleave: global-head attention units between local
                # projection halves so the in-order PE queue always has
                # independent matmuls (keeps HAM warm)
                lp = [(n, half) for n in ["wlq", "wlk1", "wlk2", "wlv"]
                      for half in range(2)]
                wsbs = {}
                for i, (hh, ic) in enumerate(
                        [(h, c) for h in range(2) for c in range(4)]):
                    rs = slice(hh * 64, (hh + 1) * 64)
                    attention_ic(hh, projT["wk"][rs], projT["wq"][rs],
                                 gvaug[hh], False, ic)
                    n, half = lp[i]
                    if half == 0:
                        wsbs[n] = emit_wdma(n, iop)
                    emit_proj_half(n, wsbs[n], hidT, half)

            # ---------- phase B: local heads (stage-interleaved so
            # the PE never idles long enough to go HAM-cold) ----------
            with tc.tile_pool(name="wp_l", bufs=1) as wp_l:
                st2 = local_prep(2, wp_l)
                st3 = local_prep(3, wp_l)
                local_pass1(2, st2, wp_l)
                local_pass1(3, st3, wp_l)
                for ic in range(4):
                    attention_ic(2, st2["k2aug"], st2["qaug"], st2["vaug"],
                                 True, ic)
                for ic in range(4):
                    attention_ic(3, st3["k2aug"], st3["qaug"], st3["vaug"],
                                 True, ic)

    nc.compile()
    return nc


def _patch_ldw_opt():
    # walrus ships with the LDWEIGHTS optimizer disabled; fp32 matmuls
    # pay a bundled weight reload per matmul, so try enabling the
    # optimizer (verified against the reference output by the caller).
    from concourse import bass_utils
    if getattr(bass_utils, "_ldw_patched", False):
        return
    orig = bass_utils.bir_verify_and_optimise

    def patched(*a, **k):
        import subprocess
        orig_run = bass_utils.run_command

        def run2(cmd, **kw):
            cmd = [c.replace("--enable-ldw-opt=false",
                             "--enable-ldw-opt=true") for c in cmd]
            return orig_run(cmd, **kw)

        bass_utils.run_command = run2
        try:
            return orig(*a, **k)
        finally:
            bass_utils.run_command = orig_run

    bass_utils.bir_verify_and_optimise = patched
    bass_utils._ldw_patched = True


def kernel(**inputs):
    from concourse import bass_utils

    if os.environ.get("LDW_OPT", "0") == "1":
        _patch_ldw_opt()

    global LAST_RESULTS
    if "nc" not in _CACHE:
        _CACHE["nc"] = _build()
    nc = _CACHE["nc"]

    inputs = dict(inputs)
    inputs["wlv"] = np.asarray(inputs["wlv1"]) + np.asarray(inputs["wlv2"])
    inputs["blv"] = np.asarray(inputs["blv1"]) + np.asarray(inputs["blv2"])
    hs = np.ascontiguousarray(np.asarray(inputs["hidden_states"], np.float32))
    am = np.ascontiguousarray(np.asarray(inputs["attention_mask"], np.float32))
    in_maps = []
    for c in range(N_CORES):
        b, g = c // 4, c % 4
        csl = slice(128 * g, 128 * (g + 1))
        m = {"hid": np.ascontiguousarray(hs[b].T), "mask": am[b, 0, 0]}
        for n in W_NAMES:
            m[n] = np.ascontiguousarray(
                np.asarray(inputs[n], np.float32)[:, csl])
            m["b" + n[1:]] = np.ascontiguousarray(
                np.asarray(inputs["b" + n[1:]], np.float32)[csl])
        in_maps.append(m)

    res = bass_utils.run_bass_kernel_spmd(
        nc, in_maps, list(range(N_CORES)),
        tmpdir=os.environ.get("BASS_TMPDIR"))
    LAST_RESULTS = res

    out = np.zeros((B, S, HID), np.float32)
    for c in range(N_CORES):
        b, g = c // 4, c % 4
        o = res.results[c]["out"]
        out[b, :, 128 * g:128 * (g + 1)] = o[:, :128]
        out[b, :, 512 + 128 * g:512 + 128 * (g + 1)] = o[:, 128:]
    return out



# revision 16
# speedup vs baseline: 1.2345x; 1.2345x over previous
"""Trainium2 Bass kernel for nn_MixedAttention.

Full inputs in, full output out. Sharding: 8 cores = 2 (batch) x 4 (head
pairs). Each core computes 2 global + 2 local heads for one batch element.

Key algebraic rewrite for the local branch:
    lscores = (lq@lk1^T)@(lk1@lk2^T) = lq @ (lk1^T@lk1) @ lk2^T
with M = lk1^T@lk1 a [64,64] matrix -- turns a 2048^3 matmul chain into
two small matmuls plus one S x S matmul (30x less PE work).

Dtype strategy: fp32 matmuls cost 4 cycles/row on the PE, float32r
(TF32-like, ~13-bit mantissa) costs 1 at free-size >= 512. Measured
rel-err budget is 2e-2 and a numerical simulation of 13-bit input
rounding through the local score chain lands at ~1.4e-3, so EVERY
matmul input here is f32r (storage is bit-identical to fp32; the PE
rounds internally). hidden/weights are declared f32r straight from
DRAM so no conversion copies exist anywhere.

Layout: scores are computed transposed st[j, i] = K_eff @ Q_eff^T so the
context matmul needs no transposed probs (lhsT = v_nat, rhs = e). v gets
an extra ones column so the softmax denominator falls out of the context
matmul for free. Global heads skip max subtraction entirely (mask folded
into the Exp bias); local heads get an exact row max from a pass-1 f32r
matmul sweep in the untransposed orientation (free-dim reduce_max), and
the -max correction rides an extra contraction row (K=65) in pass 2.

Schedule: phase A1 = input DMA (split fine-grained over 4 trigger
queues) + all 7 projections with 4-wide PSUM accumulation. Phase A2 =
global attention pair-units interleaved with local-prep and the pass-1
max sweep (hides pass-1's DVE reduces under global-attention PE work).
Phase B = local pass-2 attention. Attention works in ic-pairs: one
[128,1024] two-bank PSUM st tile per j-block, one Exp activation per
j-block, ctx accumulated in a [65,1024] two-bank PSUM tile.
"""

import math
import os
import sys

import numpy as np

sys.path.insert(0, "/opt/trn_rl_repo")

B, S, HID, HEAD = 2, 2048, 1024, 64
SC = S // 128  # 16 s-chunks of 128
HC = HID // 128  # 8 hidden chunks
N_CORES = 8
SCALE = 1.0 / math.sqrt(HEAD)

W_NAMES = ["wq", "wk", "wv", "wlq", "wlk1", "wlk2", "wlv"]

_CACHE = {}
LAST_RESULTS = None  # stash of BassKernelResults for test.py profiling


def _build():
    import concourse.mybir as mybir
    import concourse.tile as tile
    from concourse import bacc
    from concourse.masks import make_identity

    f32 = mybir.dt.float32
    f32r = mybir.dt.float32r
    AF = mybir.ActivationFunctionType
    ALU = mybir.AluOpType
    AX = mybir.AxisListType

    nc = bacc.Bacc("TRN2", target_bir_lowering=False, debug=False,
                   enable_asserts=False)

    hid_d = nc.dram_tensor("hid", (HID, S), f32r, kind="ExternalInput").ap()
    mask_d = nc.dram_tensor("mask", (S,), f32, kind="ExternalInput").ap()
    w_d = {n: nc.dram_tensor(n, (HID, 128), f32r, kind="ExternalInput").ap()
           for n in W_NAMES}
    b_d = {n: nc.dram_tensor("b" + n[1:], (128,), f32,
                             kind="ExternalInput").ap() for n in W_NAMES}
    out_d = nc.dram_tensor("out", (S, 256), f32, kind="ExternalOutput").ap()

    dma_engines = None  # set inside the TileContext

    def dma_rr(i):
        return dma_engines[i % len(dma_engines)]

    with tile.TileContext(nc) as tc:
        dma_engines = [nc.sync, nc.gpsimd, nc.scalar]
        with (
            tc.tile_pool(name="const", bufs=1) as constp,
            tc.tile_pool(name="persist", bufs=1) as pp,
            tc.tile_pool(name="ps_mm", bufs=2, space="PSUM") as ps_mm,
            tc.tile_pool(name="ps_tr", bufs=2, space="PSUM") as ps_tr,
            tc.tile_pool(name="ps_ctx", bufs=1, space="PSUM") as ps_ctx,
            tc.tile_pool(name="dramp", bufs=2, space="DRAM") as dramp,
        ):
            ident = constp.tile([128, 128], f32, name="ident")
            make_identity(nc, ident)
            identr = constp.tile([128, 128], f32r, name="identr")
            nc.vector.tensor_copy(identr, ident)
            ones_sb = constp.tile([128, SC], f32, name="ones_sb")
            nc.vector.memset(ones_sb, 1.0)
            ones_row = constp.tile([1, S], f32, name="ones_row")
            nc.vector.memset(ones_row, 1.0)
            mask_sb = constp.tile([128, SC], f32, name="mask_sb")
            nc.gpsimd.dma_start(mask_sb,
                                mask_d.rearrange("(c p) -> p c", p=128))
            bias_sb = {}
            for n in W_NAMES:
                t = constp.tile([128, 1], f32, name=f"b_{n}")
                nc.gpsimd.dma_start(t, b_d[n][:, None])
                bias_sb[n] = t

            # local-branch projections persist into phase B
            projT = {n: pp.tile([128, S], f32r, name=f"projT_{n}")
                     for n in ["wlq", "wlk1", "wlk2", "wlv"]}

            # assigned when the A2/B pools open (the helpers below only
            # run after that)
            vp = ep = op_ = out_sb = None

            # ---------- emission helpers ----------

            def build_vaug(vT):
                # v natural [s, d] + ones column -> [128, SC, 65] f32r
                base = vT.base_partition()
                idsl = slice(base, base + 64)
                vaug = vp.tile([128, SC, 65], f32r, tag="vaug",
                               name="vaug", bufs=4)
                nc.vector.tensor_copy(vaug[:, :, 64], ones_sb)
                for t in range(SC):
                    pt = ps_tr.tile([128, 128], f32r, tag="tr")
                    nc.tensor.transpose(
                        pt[:, :64], vT[:, t * 128:(t + 1) * 128],
                        identr[idsl, idsl])
                    nc.vector.tensor_copy(vaug[:, t, :64], pt[:, :64])
                return vaug

            def attention_pair(head, kT, qT, vaug, is_local, jp):
                # pair-unit: 2 i-column blocks of 512; st -> exp -> ctx
                # (+denominators via the ones column), then transpose back
                # and divide by the sums
                csl = slice(head * 64, (head + 1) * 64)
                i0 = jp * 1024
                ctx = ps_ctx.tile([65, 1024], f32, tag="ctx", name="ctx")

                def ctx_mm(jc, e):
                    nc.tensor.matmul(ctx[:, 0:512], lhsT=vaug[:, jc],
                                     rhs=e[:, 0:512],
                                     start=(jc == 0), stop=(jc == SC - 1))
                    nc.tensor.matmul(ctx[:, 512:1024], lhsT=vaug[:, jc],
                                     rhs=e[:, 512:1024],
                                     start=(jc == 0), stop=(jc == SC - 1))

                # one-jc software pipeline lag: the ctx matmuls for jc are
                # emitted after st/exp of jc+1, so the in-order PE queue
                # never waits on the Exp of the tile it just produced
                prev = None
                for jc in range(SC):
                    jsl = slice(jc * 128, (jc + 1) * 128)
                    st = ps_mm.tile([128, 1024], f32, tag="mm", name="st")
                    nc.tensor.matmul(st[:, 0:512], lhsT=kT[:, jsl],
                                     rhs=qT[:, i0:i0 + 512],
                                     start=True, stop=True)
                    nc.tensor.matmul(st[:, 512:1024], lhsT=kT[:, jsl],
                                     rhs=qT[:, i0 + 512:i0 + 1024],
                                     start=True, stop=True)
                    e = ep.tile([128, 1024], f32r, tag="e", name="e")
                    bias = 0.0 if is_local else mask_sb[:, jc:jc + 1]
                    nc.scalar.activation(e, st, AF.Exp, bias=bias,
                                         scale=SCALE)
                    if prev is not None:
                        ctx_mm(*prev)
                    prev = (jc, e)
                ctx_mm(*prev)
                ctx_sbc = vp.tile([65, 1024], f32, tag="ctx_sbc",
                                  name="ctx_sbc", bufs=2)
                nc.scalar.copy(ctx_sbc, ctx)
                for tg in range(2):
                    pts = ps_mm.tile([128, 1024], f32, tag="mm", name="pts")
                    rec = vp.tile([128, 4], f32, tag="rec", name="rec",
                                  bufs=2)
                    for q in range(4):
                        tt = tg * 4 + q
                        nc.tensor.transpose(
                            pts[:, q * 256:q * 256 + 65],
                            ctx_sbc[:, tt * 128:(tt + 1) * 128],
                            ident[:65, :65])
                        nc.vector.reciprocal(
                            rec[:, q:q + 1],
                            pts[:, q * 256 + 64:q * 256 + 65])
                    for q in range(4):
                        tt = tg * 4 + q
                        t_abs = jp * 8 + tt
                        nc.vector.tensor_scalar_mul(
                            out_sb[:, t_abs, csl],
                            pts[:, q * 256:q * 256 + 64], rec[:, q:q + 1])
                nc.gpsimd.dma_start(
                    out_d.rearrange("(t p) c -> p t c", p=128)[
                        :, jp * 8:(jp + 1) * 8, csl],
                    out_sb[:, jp * 8:(jp + 1) * 8, csl])

            def local_prep(head):
                hh = head % 2
                rs = slice(hh * 64, (hh + 1) * 64)
                idsl = slice(rs.start, rs.start + 64)
                if hh == 0:
                    lqT = projT["wlq"][rs]
                else:
                    # matmul operands must share a base partition and the
                    # PSUM dst must sit at partition 0, so head hh=1's lq
                    # is staged down to base 0 (ACT handles the shift)
                    lqT = vp.tile([64, S], f32r, tag="lqT", name="lqT",
                                  bufs=1)
                    nc.scalar.copy(lqT, projT["wlq"][rs])
                lk1T = projT["wlk1"][rs]

                # lk1 natural [s, d] via transposes (identity block at the
                # source base partition avoids any staging copy)
                lk1nat = vp.tile([128, SC, 64], f32r, tag="lk1nat",
                                 name="lk1nat", bufs=2)
                for t in range(SC):
                    pt = ps_tr.tile([128, 128], f32r, tag="tr")
                    nc.tensor.transpose(
                        pt[:, :64], lk1T[:, t * 128:(t + 1) * 128],
                        identr[idsl, idsl])
                    nc.vector.tensor_copy(lk1nat[:, t], pt[:, :64])
                # M = lk1^T @ lk1 [64, 64] (symmetric)
                mps = ps_mm.tile([128, 1024], f32, tag="mm", name="mps")
                for t in range(SC):
                    nc.tensor.matmul(mps[:64, :64], lhsT=lk1nat[:, t],
                                     rhs=lk1nat[:, t],
                                     start=(t == 0), stop=(t == SC - 1))
                m_sb = vp.tile([64, 64], f32r, tag="m_sb", name="m_sb",
                               bufs=2)
                nc.vector.tensor_copy(m_sb, mps[:64, :64])
                # qaug rows 0:64 = (lq @ M)^T = M @ lq^T (M symmetric);
                # row 64 filled later with -max
                qaug = vp.tile([65, S], f32r, tag="qaug", name="qaug",
                               bufs=2)
                for half in range(2):
                    mm = ps_mm.tile([128, 1024], f32, tag="mm", name="mm")
                    for ic in range(2):
                        icg = half * 2 + ic
                        nc.tensor.matmul(
                            mm[:64, ic * 512:(ic + 1) * 512], lhsT=m_sb,
                            rhs=lqT[:, icg * 512:(icg + 1) * 512],
                            start=True, stop=True)
                    nc.vector.tensor_copy(
                        qaug[:64, half * 1024:(half + 1) * 1024], mm[:64])
                # k2aug: rows 0:64 = lk2^T, row 64 = ones
                k2aug = vp.tile([65, S], f32r, tag="k2aug", name="k2aug",
                                bufs=2)
                nc.scalar.copy(k2aug[:64, :], projT["wlk2"][rs])
                nc.vector.tensor_copy(k2aug[64:65, :], ones_row)
                vaug = build_vaug(projT["wlv"][rs])
                pmax = vp.tile([128, SC, 2], f32, tag="pmax", name="pmax",
                               bufs=2)
                return dict(qaug=qaug, k2aug=k2aug, vaug=vaug, pmax=pmax)

            def pass1_unit(hs, t):
                # one t-block of the pass-1 max sweep: raw scores in the
                # untransposed orientation, row max via free-dim reduce
                qaug, k2aug, pmax = hs["qaug"], hs["k2aug"], hs["pmax"]
                tsl = slice(t * 128, (t + 1) * 128)
                for jp in range(2):
                    st = ps_mm.tile([128, 1024], f32, tag="mm", name="st1")
                    for j2 in range(2):
                        j0 = jp * 1024 + j2 * 512
                        nc.tensor.matmul(st[:, j2 * 512:(j2 + 1) * 512],
                                         lhsT=qaug[:64, tsl],
                                         rhs=k2aug[:64, j0:j0 + 512],
                                         start=True, stop=True)
                    nc.vector.tensor_reduce(pmax[:, t, jp:jp + 1], st,
                                            axis=AX.X, op=ALU.max)

            def pass1_finish(hs):
                # combine pair maxes, negate, and route [128, SC] -> [1, S]
                # via a DRAM roundtrip into qaug row 64
                maxneg = vp.tile([128, SC], f32, tag="maxneg",
                                 name="maxneg", bufs=2)
                nc.vector.tensor_reduce(maxneg, hs["pmax"], axis=AX.X,
                                        op=ALU.max, negate=True)
                mscr = dramp.tile([S], f32, tag="mscr", name="mscr")
                nc.sync.dma_start(
                    mscr.rearrange("(t p) -> p t", p=128), maxneg)
                nc.gpsimd.dma_start(hs["qaug"][64:65, :], mscr[None, :])

            # ---------- phase A1: hidden^T + all 7 projections ----------
            with tc.tile_pool(name="pp_g", bufs=1) as pp_g:
                for n in ["wq", "wk", "wv"]:
                    projT[n] = pp_g.tile([128, S], f32r, name=f"projT_{n}")

                with (
                    tc.tile_pool(name="hidT", bufs=1) as hp,
                    tc.tile_pool(name="io", bufs=4) as iop,
                ):
                    hidT = hp.tile([128, HC, S], f32r, name="hidT")
                    hid_r = hid_d.rearrange("(c p) s -> p c s", p=128)
                    dmai = 0
                    wsbs = {}

                    def emit_wdma(n):
                        nonlocal dmai
                        wsb = iop.tile([128, HC, 128], f32r, tag="w",
                                       name=f"w_{n}")
                        dma_rr(dmai).dma_start(
                            wsb, w_d[n].rearrange("(c p) m -> p c m", p=128))
                        dmai += 1
                        wsbs[n] = wsb

                    for n in W_NAMES[:3]:
                        emit_wdma(n)
                    for hc in range(HC):
                        for icq in range(4):
                            ssl = slice(icq * 512, (icq + 1) * 512)
                            dma_rr(dmai).dma_start(hidT[:, hc, ssl],
                                                   hid_r[:, hc, ssl])
                            dmai += 1

                    def emit_proj(n):
                        accs = [ps_mm.tile([128, 1024], f32, tag="mm",
                                           name=f"acc{i}") for i in range(2)]
                        for hc in range(HC):
                            for ic in range(4):
                                nc.tensor.matmul(
                                    accs[ic // 2][:, (ic % 2) * 512:
                                                  (ic % 2 + 1) * 512],
                                    lhsT=wsbs[n][:, hc],
                                    rhs=hidT[:, hc, ic * 512:(ic + 1) * 512],
                                    start=(hc == 0), stop=(hc == HC - 1))
                        for i in range(2):
                            nc.vector.tensor_scalar_add(
                                projT[n][:, i * 1024:(i + 1) * 1024],
                                accs[i], bias_sb[n])

                    for pi, n in enumerate(W_NAMES):
                        if pi + 3 < len(W_NAMES):
                            emit_wdma(W_NAMES[pi + 3])
                        emit_proj(n)

                # ---------- phase A2: global attention + local prep +
                # pass-1 max sweep (interleaved) ----------
                with (
                    tc.tile_pool(name="vpool", bufs=1) as vp,
                    tc.tile_pool(name="epool", bufs=4) as ep,
                    tc.tile_pool(name="opool", bufs=1) as op_,
                ):
                    out_sb = op_.tile([128, SC, 256], f32, name="out_sb")
                    gvaug = {hh: build_vaug(
                        projT["wv"][hh * 64:(hh + 1) * 64])
                        for hh in range(2)}
                    st2 = local_prep(2)
                    st3 = local_prep(3)

                    for u, (hh, jp) in enumerate(
                            [(h, p) for h in range(2) for p in range(2)]):
                        rs = slice(hh * 64, (hh + 1) * 64)
                        attention_pair(hh, projT["wk"][rs], projT["wq"][rs],
                                       gvaug[hh], False, jp)
                        hs = st2 if u < 2 else st3
                        for t in range(8):
                            pass1_unit(hs, (u % 2) * 8 + t)
                        if u % 2 == 1:
                            pass1_finish(hs)

                    # ---------- phase B: local pass-2 attention ----------
                    for head, hs in ((2, st2), (3, st3)):
                        for jp in range(2):
                            attention_pair(head, hs["k2aug"], hs["qaug"],
                                           hs["vaug"], True, jp)

    nc.compile()
    return nc


def kernel(**inputs):
    from concourse import bass_utils

    global LAST_RESULTS
    if "nc" not in _CACHE:
        _CACHE["nc"] = _build()
    nc = _CACHE["nc"]

    inputs = dict(inputs)
    inputs["wlv"] = np.asarray(inputs["wlv1"]) + np.asarray(inputs["wlv2"])
    inputs["blv"] = np.asarray(inputs["blv1"]) + np.asarray(inputs["blv2"])
    hs = np.ascontiguousarray(np.asarray(inputs["hidden_states"], np.float32))
    am = np.ascontiguousarray(np.asarray(inputs["attention_mask"], np.float32))
    in_maps = []
    for c in range(N_CORES):
        b, g = c // 4, c % 4
        csl = slice(128 * g, 128 * (g + 1))
        m = {"hid": np.ascontiguousarray(hs[b].T), "mask": am[b, 0, 0]}
        for n in W_NAMES:
            m[n] = np.ascontiguousarray(
                np.asarray(inputs[n], np.float32)[:, csl])
            m["b" + n[1:]] = np.ascontiguousarray(
                np.asarray(inputs["b" + n[1:]], np.float32)[csl])
        in_maps.append(m)

    res = bass_utils.run_bass_kernel_spmd(
        nc, in_maps, list(range(N_CORES)),
        tmpdir=os.environ.get("BASS_TMPDIR"))
    LAST_RESULTS = res

    out = np.zeros((B, S, HID), np.float32)
    for c in range(N_CORES):
        b, g = c // 4, c % 4
        o = res.results[c]["out"]
        out[b, :, 128 * g:128 * (g + 1)] = o[:, :128]
        out[b, :, 512 + 128 * g:512 + 128 * (g + 1)] = o[:, 128:]
    return out


# revision 23
# speedup vs baseline: 1.5612x; 1.2646x over previous
"""Trainium2 Bass kernel for nn_MixedAttention.

Full inputs in, full output out. Sharding: 8 cores = 2 (batch) x 4 (head
pairs). Each core computes 2 global + 2 local heads for one batch element.

Key algebraic rewrite for the local branch:
    lscores = (lq@lk1^T)@(lk1@lk2^T) = lq @ (lk1^T@lk1) @ lk2^T
with M = lk1^T@lk1 a [64,64] matrix -- turns a 2048^3 matmul chain into
two small matmuls plus one S x S matmul (30x less PE work).

Dtype strategy: fp32 matmuls cost 4 cycles/row on the PE, float32r
(TF32-like, ~13-bit mantissa) costs 1 at free-size >= 512. Measured
rel-err budget is 2e-2 and a numerical simulation of 13-bit input
rounding through the local score chain lands at ~1.4e-3, so EVERY
matmul input here is f32r (storage is bit-identical to fp32; the PE
rounds internally). hidden/weights are declared f32r straight from
DRAM so no conversion copies exist anywhere.

Layout: scores are computed transposed st[j, i] = K_eff @ Q_eff^T so the
context matmul needs no transposed probs (lhsT = v_nat, rhs = e). v gets
an extra ones column so the softmax denominator falls out of the context
matmul for free. Global heads skip max subtraction entirely (mask folded
into the Exp bias); local heads get an exact row max from a pass-1 f32r
matmul sweep in the untransposed orientation (free-dim reduce_max), and
the -max correction rides an extra contraction row (K=65) in pass 2.

Schedule: phase A1 = input DMA (split fine-grained over 4 trigger
queues) + all 7 projections with 4-wide PSUM accumulation. Phase A2 =
global attention pair-units interleaved with local-prep and the pass-1
max sweep (hides pass-1's DVE reduces under global-attention PE work).
Phase B = local pass-2 attention. Attention works in ic-pairs: one
[128,1024] two-bank PSUM st tile per j-block, one Exp activation per
j-block, ctx accumulated in a [65,1024] two-bank PSUM tile.
"""

import math
import os
import sys

import numpy as np

sys.path.insert(0, "/opt/trn_rl_repo")

B, S, HID, HEAD = 2, 2048, 1024, 64
SC = S // 128  # 16 s-chunks of 128
HC = HID // 128  # 8 hidden chunks
N_CORES = 8
SCALE = 1.0 / math.sqrt(HEAD)

W_NAMES = ["wq", "wk", "wv", "wlq", "wlk1", "wlk2", "wlv"]

_CACHE = {}
LAST_RESULTS = None  # stash of BassKernelResults for test.py profiling


def _build():
    import concourse.mybir as mybir
    import concourse.tile as tile
    from concourse import bacc
    from concourse.masks import make_identity

    f32 = mybir.dt.float32
    f32r = mybir.dt.float32r
    bf16 = mybir.dt.bfloat16
    AF = mybir.ActivationFunctionType
    ALU = mybir.AluOpType
    AX = mybir.AxisListType

    nc = bacc.Bacc("TRN2", target_bir_lowering=False, debug=False,
                   enable_asserts=False)

    hid_d = nc.dram_tensor("hid", (HID, S), f32r, kind="ExternalInput").ap()
    mask_d = nc.dram_tensor("mask", (S,), f32, kind="ExternalInput").ap()
    w_d = {n: nc.dram_tensor(n, (HID, 128), f32r, kind="ExternalInput").ap()
           for n in W_NAMES}
    b_d = {n: nc.dram_tensor("b" + n[1:], (128,), f32,
                             kind="ExternalInput").ap() for n in W_NAMES}
    out_d = nc.dram_tensor("out", (S, 256), f32, kind="ExternalOutput").ap()

    dma_engines = None  # set inside the TileContext

    def dma_rr(i):
        return dma_engines[i % len(dma_engines)]

    with tile.TileContext(nc) as tc:
        dma_engines = [nc.sync, nc.gpsimd, nc.scalar]
        with (
            tc.tile_pool(name="const", bufs=1) as constp,
            tc.tile_pool(name="persist", bufs=1) as pp,
            tc.tile_pool(name="ps_mm", bufs=3, space="PSUM") as ps_mm,
            tc.tile_pool(name="ps_ctx", bufs=1, space="PSUM") as ps_ctx,
            tc.tile_pool(name="dramp", bufs=2, space="DRAM") as dramp,
        ):
            ident = constp.tile([128, 128], f32, name="ident")
            make_identity(nc, ident)
            ones_sb = constp.tile([128, SC], f32, name="ones_sb")
            nc.vector.memset(ones_sb, 1.0)
            ones_row = constp.tile([1, S], f32, name="ones_row")
            nc.vector.memset(ones_row, 1.0)
            mask_sb = constp.tile([128, SC], f32, name="mask_sb")
            nc.gpsimd.dma_start(mask_sb,
                                mask_d.rearrange("(c p) -> p c", p=128))
            bias_sb = {}
            for n in W_NAMES:
                t = constp.tile([128, 1], f32, name=f"b_{n}")
                nc.gpsimd.dma_start(t, b_d[n][:, None])
                bias_sb[n] = t

            # local-branch projections persist into phase B
            projT = {n: pp.tile([128, S],
                                f32 if n in ("wlk1", "wlv") else f32r,
                                name=f"projT_{n}")
                     for n in ["wlq", "wlk1", "wlk2", "wlv"]}

            # assigned when the A2/B pools open (the helpers below only
            # run after that)
            vp = ep = op_ = out_sb = None

            # ---------- emission helpers ----------

            def build_vaug(vT):
                # v natural [s, d] + ones column -> [128, SC, 65] bf16
                # (bf16 halves the per-ctx-matmul LDWEIGHTS stream; value
                # path tolerates the 8-bit mantissa)
                base = vT.base_partition()
                idsl = slice(base, base + 64)
                vaug = vp.tile([128, SC, 65], bf16, tag="vaug",
                               name="vaug", bufs=4)
                nc.vector.tensor_copy(vaug[:, :, 64], ones_sb)
                for t in range(SC):
                    pt = ps_mm.tile([128, 1024], f32, tag="mm", name="ptv")
                    nc.tensor.transpose(
                        pt[:, :64], vT[:, t * 128:(t + 1) * 128],
                        ident[idsl, idsl])
                    nc.vector.tensor_copy(vaug[:, t, :64], pt[:, :64])
                return vaug

            def attention_pair(head, kT, qT, vaug, is_local, jp,
                               filler=None):
                # pair-unit: 2 i-column blocks of 512; st -> exp -> ctx
                # (+denominators via the ones column), then transpose back
                # and divide by the sums. `filler` emits ACT-independent PE
                # work each jc so the tensor engine never starves while the
                # Exp runs (starvation resets the PE's DVFS ramp).
                csl = slice(head * 64, (head + 1) * 64)
                i0 = jp * 1024
                ctx = ps_ctx.tile([65, 1024], f32, tag="ctx", name="ctx")

                def ctx_mm(jc, e):
                    nc.tensor.matmul(ctx[:, 0:512], lhsT=vaug[:, jc],
                                     rhs=e[:, 0:512],
                                     start=(jc == 0), stop=(jc == SC - 1))
                    nc.tensor.matmul(ctx[:, 512:1024], lhsT=vaug[:, jc],
                                     rhs=e[:, 512:1024],
                                     start=(jc == 0), stop=(jc == SC - 1))

                # one-jc software pipeline lag: the ctx matmuls for jc are
                # emitted after st/exp of jc+1, so the in-order PE queue
                # never waits on the Exp of the tile it just produced
                prev = None
                for jc in range(SC):
                    jsl = slice(jc * 128, (jc + 1) * 128)
                    st = ps_mm.tile([128, 1024], f32, tag="mm", name="st")
                    nc.tensor.matmul(st[:, 0:512], lhsT=kT[:, jsl],
                                     rhs=qT[:, i0:i0 + 512],
                                     start=True, stop=True)
                    nc.tensor.matmul(st[:, 512:1024], lhsT=kT[:, jsl],
                                     rhs=qT[:, i0 + 512:i0 + 1024],
                                     start=True, stop=True)
                    e = ep.tile([128, 1024], bf16, tag="e", name="e")
                    bias = 0.0 if is_local else mask_sb[:, jc:jc + 1]
                    nc.scalar.activation(e, st, AF.Exp, bias=bias,
                                         scale=SCALE)
                    if filler is not None:
                        filler()
                    if prev is not None:
                        ctx_mm(*prev)
                    prev = (jc, e)
                ctx_mm(*prev)
                ctx_sbc = vp.tile([65, 1024], f32, tag="ctx_sbc",
                                  name="ctx_sbc", bufs=2)
                nc.scalar.copy(ctx_sbc, ctx)
                for tg in range(2):
                    pts = ps_mm.tile([128, 1024], f32, tag="mm", name="pts")
                    rec = vp.tile([128, 4], f32, tag="rec", name="rec",
                                  bufs=2)
                    for q in range(4):
                        tt = tg * 4 + q
                        nc.tensor.transpose(
                            pts[:, q * 256:q * 256 + 65],
                            ctx_sbc[:, tt * 128:(tt + 1) * 128],
                            ident[:65, :65])
                        nc.vector.reciprocal(
                            rec[:, q:q + 1],
                            pts[:, q * 256 + 64:q * 256 + 65])
                    for q in range(4):
                        tt = tg * 4 + q
                        t_abs = jp * 8 + tt
                        nc.vector.tensor_scalar_mul(
                            out_sb[:, t_abs, csl],
                            pts[:, q * 256:q * 256 + 64], rec[:, q:q + 1])
                nc.gpsimd.dma_start(
                    out_d.rearrange("(t p) c -> p t c", p=128)[
                        :, jp * 8:(jp + 1) * 8, csl],
                    out_sb[:, jp * 8:(jp + 1) * 8, csl])

            def local_prep(head):
                hh = head % 2
                rs = slice(hh * 64, (hh + 1) * 64)
                idsl = slice(rs.start, rs.start + 64)
                if hh == 0:
                    lqT = projT["wlq"][rs]
                else:
                    # matmul operands must share a base partition and the
                    # PSUM dst must sit at partition 0, so head hh=1's lq
                    # is staged down to base 0 (ACT handles the shift)
                    lqT = vp.tile([64, S], f32r, tag="lqT", name="lqT",
                                  bufs=1)
                    nc.scalar.copy(lqT, projT["wlq"][rs])
                lk1T = projT["wlk1"][rs]

                # lk1 natural [s, d] via transposes (identity block at the
                # source base partition avoids any staging copy)
                lk1nat = vp.tile([128, SC, 64], f32r, tag="lk1nat",
                                 name="lk1nat", bufs=2)
                for t in range(SC):
                    pt = ps_mm.tile([128, 1024], f32, tag="mm", name="ptk")
                    nc.tensor.transpose(
                        pt[:, :64], lk1T[:, t * 128:(t + 1) * 128],
                        ident[idsl, idsl])
                    nc.vector.tensor_copy(lk1nat[:, t], pt[:, :64])
                # M = lk1^T @ lk1 [64, 64] (symmetric)
                mps = ps_mm.tile([128, 1024], f32, tag="mm", name="mps")
                for t in range(SC):
                    nc.tensor.matmul(mps[:64, :64], lhsT=lk1nat[:, t],
                                     rhs=lk1nat[:, t],
                                     start=(t == 0), stop=(t == SC - 1))
                m_sb = vp.tile([64, 64], f32r, tag="m_sb", name="m_sb",
                               bufs=2)
                nc.vector.tensor_copy(m_sb, mps[:64, :64])
                # qaug rows 0:64 = (lq @ M)^T = M @ lq^T (M symmetric);
                # row 64 filled later with -max
                qaug = vp.tile([65, S], f32r, tag="qaug", name="qaug",
                               bufs=2)
                for half in range(2):
                    mm = ps_mm.tile([128, 1024], f32, tag="mm", name="mm")
                    for ic in range(2):
                        icg = half * 2 + ic
                        nc.tensor.matmul(
                            mm[:64, ic * 512:(ic + 1) * 512], lhsT=m_sb,
                            rhs=lqT[:, icg * 512:(icg + 1) * 512],
                            start=True, stop=True)
                    nc.vector.tensor_copy(
                        qaug[:64, half * 1024:(half + 1) * 1024], mm[:64])
                # k2aug: rows 0:64 = lk2^T, row 64 = ones
                k2aug = vp.tile([65, S], f32r, tag="k2aug", name="k2aug",
                                bufs=2)
                nc.scalar.copy(k2aug[:64, :], projT["wlk2"][rs])
                nc.vector.tensor_copy(k2aug[64:65, :], ones_row)
                vaug = build_vaug(projT["wlv"][rs])
                pmax = vp.tile([128, SC, 2], f32, tag="pmax", name="pmax",
                               bufs=2)
                return dict(qaug=qaug, k2aug=k2aug, vaug=vaug, pmax=pmax)

            def pass1_pair(hs, t, jp):
                # one (t, jp) unit of the pass-1 max sweep: raw scores in
                # the untransposed orientation, row max via free-dim reduce
                qaug, k2aug, pmax = hs["qaug"], hs["k2aug"], hs["pmax"]
                tsl = slice(t * 128, (t + 1) * 128)
                st = ps_mm.tile([128, 1024], f32, tag="mm", name="st1")
                for j2 in range(2):
                    j0 = jp * 1024 + j2 * 512
                    nc.tensor.matmul(st[:, j2 * 512:(j2 + 1) * 512],
                                     lhsT=qaug[:64, tsl],
                                     rhs=k2aug[:64, j0:j0 + 512],
                                     start=True, stop=True)
                nc.vector.tensor_reduce(pmax[:, t, jp:jp + 1], st,
                                        axis=AX.X, op=ALU.max)

            def make_filler(units):
                it = iter(units)

                def filler():
                    u = next(it, None)
                    if u is not None:
                        pass1_pair(*u)
                return filler

            def pass1_finish(hs):
                # combine pair maxes, negate, and route [128, SC] -> [1, S]
                # via a DRAM roundtrip into qaug row 64
                maxneg = vp.tile([128, SC], f32, tag="maxneg",
                                 name="maxneg", bufs=2)
                nc.vector.tensor_reduce(maxneg, hs["pmax"], axis=AX.X,
                                        op=ALU.max, negate=True)
                mscr = dramp.tile([S], f32, tag="mscr", name="mscr")
                nc.sync.dma_start(
                    mscr.rearrange("(t p) -> p t", p=128), maxneg)
                nc.gpsimd.dma_start(hs["qaug"][64:65, :], mscr[None, :])

            # ---------- phase A1: hidden^T + all 7 projections ----------
            with tc.tile_pool(name="pp_g", bufs=1) as pp_g:
                for n in ["wq", "wk", "wv"]:
                    projT[n] = pp_g.tile([128, S],
                                         f32 if n == "wv" else f32r,
                                         name=f"projT_{n}")

                with (
                    tc.tile_pool(name="hidT", bufs=1) as hp,
                    tc.tile_pool(name="io", bufs=4) as iop,
                ):
                    hidT = hp.tile([128, HC, S], f32r, name="hidT")
                    hid_r = hid_d.rearrange("(c p) s -> p c s", p=128)
                    dmai = 0
                    wsbs = {}

                    def emit_wdma(n):
                        nonlocal dmai
                        wsb = iop.tile([128, HC, 128], f32r, tag="w",
                                       name=f"w_{n}")
                        dma_rr(dmai).dma_start(
                            wsb, w_d[n].rearrange("(c p) m -> p c m", p=128))
                        dmai += 1
                        wsbs[n] = wsb

                    for n in W_NAMES[:3]:
                        emit_wdma(n)
                    for hc in range(HC):
                        for icq in range(4):
                            ssl = slice(icq * 512, (icq + 1) * 512)
                            dma_rr(dmai).dma_start(hidT[:, hc, ssl],
                                                   hid_r[:, hc, ssl])
                            dmai += 1

                    def emit_proj(n):
                        accs = [ps_mm.tile([128, 1024], f32, tag="mm",
                                           name=f"acc{i}") for i in range(2)]
                        for hc in range(HC):
                            for ic in range(4):
                                nc.tensor.matmul(
                                    accs[ic // 2][:, (ic % 2) * 512:
                                                  (ic % 2 + 1) * 512],
                                    lhsT=wsbs[n][:, hc],
                                    rhs=hidT[:, hc, ic * 512:(ic + 1) * 512],
                                    start=(hc == 0), stop=(hc == HC - 1))
                        for i in range(2):
                            nc.vector.tensor_scalar_add(
                                projT[n][:, i * 1024:(i + 1) * 1024],
                                accs[i], bias_sb[n])

                    for pi, n in enumerate(W_NAMES):
                        if pi + 3 < len(W_NAMES):
                            emit_wdma(W_NAMES[pi + 3])
                        emit_proj(n)

                # ---------- phase A2: global attention + local prep +
                # pass-1 max sweep (interleaved) ----------
                with (
                    tc.tile_pool(name="vpool", bufs=1) as vp,
                    tc.tile_pool(name="epool", bufs=4) as ep,
                    tc.tile_pool(name="opool", bufs=1) as op_,
                ):
                    out_sb = op_.tile([128, SC, 256], f32, name="out_sb")
                    gvaug = {hh: build_vaug(
                        projT["wv"][hh * 64:(hh + 1) * 64])
                        for hh in range(2)}
                    st2 = local_prep(2)
                    st3 = local_prep(3)

                    f2 = make_filler(
                        [(st2, t, jp) for t in range(SC) for jp in range(2)])
                    f3 = make_filler(
                        [(st3, t, jp) for t in range(SC) for jp in range(2)])
                    for u, (hh, jp) in enumerate(
                            [(h, p) for h in range(2) for p in range(2)]):
                        rs = slice(hh * 64, (hh + 1) * 64)
                        attention_pair(hh, projT["wk"][rs], projT["wq"][rs],
                                       gvaug[hh], False, jp,
                                       filler=f2 if u < 2 else f3)
                        if u == 1:
                            pass1_finish(st2)
                        if u == 3:
                            pass1_finish(st3)

                    # ---------- phase B: local pass-2 attention ----------
                    for head, hs in ((2, st2), (3, st3)):
                        for jp in range(2):
                            attention_pair(head, hs["k2aug"], hs["qaug"],
                                           hs["vaug"], True, jp)

    nc.compile()
    return nc


def kernel(**inputs):
    from concourse import bass_utils

    global LAST_RESULTS
    if "nc" not in _CACHE:
        _CACHE["nc"] = _build()
    nc = _CACHE["nc"]

    inputs = dict(inputs)
    inputs["wlv"] = np.asarray(inputs["wlv1"]) + np.asarray(inputs["wlv2"])
    inputs["blv"] = np.asarray(inputs["blv1"]) + np.asarray(inputs["blv2"])
    hs = np.ascontiguousarray(np.asarray(inputs["hidden_states"], np.float32))
    am = np.ascontiguousarray(np.asarray(inputs["attention_mask"], np.float32))
    in_maps = []
    for c in range(N_CORES):
        b, g = c // 4, c % 4
        csl = slice(128 * g, 128 * (g + 1))
        m = {"hid": np.ascontiguousarray(hs[b].T), "mask": am[b, 0, 0]}
        for n in W_NAMES:
            m[n] = np.ascontiguousarray(
                np.asarray(inputs[n], np.float32)[:, csl])
            m["b" + n[1:]] = np.ascontiguousarray(
                np.asarray(inputs["b" + n[1:]], np.float32)[csl])
        in_maps.append(m)

    res = bass_utils.run_bass_kernel_spmd(
        nc, in_maps, list(range(N_CORES)),
        tmpdir=os.environ.get("BASS_TMPDIR"))
    LAST_RESULTS = res

    out = np.zeros((B, S, HID), np.float32)
    for c in range(N_CORES):
        b, g = c // 4, c % 4
        o = res.results[c]["out"]
        out[b, :, 128 * g:128 * (g + 1)] = o[:, :128]
        out[b, :, 512 + 128 * g:512 + 128 * (g + 1)] = o[:, 128:]
    return out
